# revision 31
# baseline (speedup 1.0000x reference)
"""Enframe (overlapping-frame unfold) kernel for Trainium2.

Math: out[b, c*FL + k, t] = x[b, c, t*HOP + k]  with FL=2048, HOP=512,
T = (S - FL)//HOP + 1 = 934.

Decomposition (k = 512*q + 128*i + p, q,i in [0,4), p in [0,128)):
    out[b, c*FL + 512q + 128i + p, t] = X[t+q, 128i+p]
where X[j, r] = x[b, c, j*512 + r] (j < 937). Per (b, c) this is one
937x512 -> 512x937 transpose; each of the 16 output row-blocks is a
contiguous column-slice XT[128i:128(i+1), q:q+934] written densely.

Schedule per core (one batch element per NeuronCore, 8-way data parallel):
  - HBM is the budget (~19.1 MB at ~390 GB/s aggregate over the three DMA
    dispatch rings). The kernel keeps every ring's FIFO non-empty from the
    first load to the last store so the HBM pipe never idles:
      * loads (2 KB descriptors) ride the gpsimd SWDGE ring, which streams
        them ~2.3x faster than a HWDGE ring (Q7 packs 64-desc packets);
        four small c0 pieces go to the idle HWDGE rings early, before the
        deep SWDGE queue can starve them of shared-engine time.
      * the 41-row remainder loads as a full 128-partition tail tile
        overlapping jc6 (a 41-descriptor DMA is chopped onto ~2 SDMA
        engines and its late semaphore would gate the whole first block).
      * transposes (TensorE via identity matmul, PSUM, DVE copies) chase
        the load stream in landing order; each block's 4 dense ~478 KB
        stores (3736 B/descriptor) alternate over the two HWDGE rings the
        moment the block is assembled, draining behind the c1 loads so
        HBM stays saturated from first load byte to final store.
  - The NEFF's fixed ~6.3 us full-semaphore-file clear epilogue + barriers
    and the ~2 us framework preamble are invariant; everything between is
    paced by HBM. Measured 63.1-63.9 us/core unthrottled; hardware
    activity-throttling adds 0-20 us on hot runs.
"""

import numpy as np

import concourse.mybir as mybir
import concourse.tile as tile
from concourse import bacc, bass_utils

B, C, S = 8, 2, 480000
FL, HOP = 2048, 512
T = (S - FL) // HOP + 1          # 934 frames
NQ = FL // HOP                   # 4 hop-shifts per frame length
NJ = T + NQ - 1                  # 937 hop-chunks of input actually used
P = 128
NI = HOP // P                    # 4 row-blocks of 128 within a hop
NJC_FULL = NJ // P               # 7 full 128-row chunks
NJ_REM = NJ - NJC_FULL * P       # 41 remainder rows
F32 = mybir.dt.float32

_NC_CACHE = None


def _emit(tc, nc, x, ident_in, out):
    # x: [C, S] f32 (this core's batch element), out: [C*FL, T] f32
    sy, sc, gp = nc.sync, nc.scalar, nc.gpsimd
    store_rr = [0]

    def store_dma(dst, src):
        eng = (sy, sc)[store_rr[0] & 1]
        store_rr[0] += 1
        eng.dma_start(dst, src)

    with tc.tile_pool(name="consts", bufs=1) as consts, \
         tc.tile_pool(name="loads", bufs=1) as loadp, \
         tc.tile_pool(name="xt", bufs=1) as xtp, \
         tc.tile_pool(name="ps", bufs=8, space="PSUM") as psp:
        ident = consts.tile([P, P], F32, name="ident")
        sy.dma_start(ident[:, :], ident_in[:, :])

        # Load layout: a_all[p, jc*HOP + r] = x[c, (jc*128 + p)*HOP + r]
        # (dense 2 KB rows per partition per jc chunk); a_tail covers the
        # last 128 hop-chunks j in [NJ-128, NJ), overlapping jc6.
        a_alls, a_tails = [], []
        for c in range(C):
            a_alls.append(
                loadp.tile([P, NJC_FULL * HOP], F32, name=f"a{c}", tag=f"a{c}")
            )
            a_tails.append(
                loadp.tile([P, HOP], F32, name=f"at{c}", tag=f"at{c}")
            )
        # The 41 remainder hop-chunks load as a full 128-partition tail
        # tile (overlapping jc6 — the duplicate read is 0.26 MB and keeps
        # every load a clean 128-descriptor DMA; a 41-descriptor DMA gets
        # chopped onto ~2 SDMA engines and its semaphore then gates the
        # whole first block). Most of c0 rides the SWDGE ring in jc order;
        # two small single-piece DMAs go to the HWDGE rings early enough
        # to finish before the deep SWDGE queue starves them (SWDGE's
        # 64-descriptor packets monopolize shared engines). c1 streams
        # behind c0 on the SWDGE ring only, keeping the HWDGE rings clear
        # for c0's stores.
        JT0 = NJ - P                           # tail covers j in [JT0, NJ)
        avs, xv_fulls = [], []
        for c in range(C):
            xv_fulls.append(
                x[c, 0:NJC_FULL * P * HOP].rearrange(
                    "(jc p r) -> p jc r", p=P, r=HOP
                )
            )
            avs.append(a_alls[c][:, :].rearrange("p (jc r) -> p jc r", r=HOP))

        def load(c, j0, j1, eng):
            eng.dma_start(avs[c][:, j0:j1], xv_fulls[c][:, j0:j1])

        def load_tail(c, eng):
            xv = x[c, 0:NJ * HOP].rearrange("(j r) -> j r", r=HOP)
            eng.dma_start(a_tails[c][:, :], xv[JT0:NJ])

        load_tail(0, gp)
        load(0, 0, 2, gp)
        load(0, 2, 3, gp)
        load(0, 3, 4, sy)
        load(0, 4, 5, sy)
        load(0, 5, 6, sc)
        load(0, 6, 7, sc)
        load_tail(1, gp)
        load(1, 0, 4, gp)
        load(1, 4, 7, gp)

        # Transpose + store. xt tiles are distinct per (c, i) so no reuse
        # dependencies gate the pipeline; each block's 4 dense ~478 KB
        # stores enqueue the moment its 8 PSUM->SBUF copies land.
        # Per-block transpose order chases load-landing order (for c0 the
        # sync/scalar pieces land first, then the tail, then the gp pieces;
        # for c1 the tail lands first, then jc0..6 stream in order).
        jc_order = {
            0: (3, 4, 5, 6, NJC_FULL, 0, 1, 2),
            1: (NJC_FULL, *range(NJC_FULL)),
        }
        for c in range(C):
            a_all, a_tail = a_alls[c], a_tails[c]
            for i in range(NI):
                xt = xtp.tile([P, NJ], F32, name=f"xt{c}{i}", tag=f"xt{c}{i}")
                for jc in jc_order[c]:
                    pt = psp.tile([P, P], F32, name="pt", tag="pt")
                    if jc < NJC_FULL:
                        j0, nj = jc * P, P
                        src = a_all[:, jc * HOP + i * P: jc * HOP + (i + 1) * P]
                        nc.tensor.transpose(pt[:, :nj], src, ident[:nj, :nj])
                        nc.vector.tensor_copy(xt[:, j0:j0 + nj], pt[:, :nj])
                    else:
                        # Remainder: transpose a 64-row slice (matmul base
                        # partition must be 0/32/64) and copy out the 41
                        # columns that land past jc6.
                        j0 = NJC_FULL * P
                        off = j0 - (JT0 + 64)
                        src = a_tail[64:P, i * P:(i + 1) * P]
                        nc.tensor.transpose(pt[:, :64], src, ident[64:P, 64:P])
                        nc.vector.tensor_copy(
                            xt[:, j0:NJ], pt[:, off:off + NJ_REM]
                        )
                for q in range(NQ):
                    base = c * FL + q * HOP + i * P
                    if c == C - 1 and i == NI - 1 and q == NQ - 1:
                        # Last store: split so the scalar ring (which
                        # otherwise trails by ~0.6 us) hands its tail to
                        # the sync ring and both drain out together.
                        ts = 660
                        store_dma(out[base:base + P, :ts], xt[:, q:q + ts])
                        store_dma(out[base:base + P, ts:], xt[:, q + ts:q + T])
                    else:
                        store_dma(out[base:base + P, :], xt[:, q:q + T])


def _build():
    nc = bacc.Bacc(
        "TRN2",
        target_bir_lowering=False,
        debug=False,
        enable_asserts=False,
        num_devices=B,
    )
    x = nc.dram_tensor("x", [C, S], F32, kind="ExternalInput").ap()
    ident_in = nc.dram_tensor("ident", [P, P], F32, kind="ExternalInput").ap()
    out = nc.dram_tensor("out", [C * FL, T], F32, kind="ExternalOutput").ap()
    with tile.TileContext(nc) as tc:
        _emit(tc, nc, x, ident_in, out)
    nc.compile()
    return nc


def _get_nc():
    global _NC_CACHE
    if _NC_CACHE is None:
        _NC_CACHE = _build()
    return _NC_CACHE


def make_in_maps(x):
    ident = np.eye(P, dtype=np.float32)
    return [
        {"x": np.ascontiguousarray(x[b]), "ident": ident} for b in range(B)
    ]


def kernel(**inputs):
    x = np.ascontiguousarray(np.asarray(inputs["x"]), dtype=np.float32)
    assert x.shape == (B, C, S), x.shape
    nc = _get_nc()
    res = bass_utils.run_bass_kernel_spmd(
        nc, make_in_maps(x), core_ids=list(range(B))
    )
    return np.stack([r["out"] for r in res.results], axis=0)


# revision 32
# speedup vs baseline: 1.0172x; 1.0172x over previous
"""Enframe (overlapping-frame unfold) kernel for Trainium2.

Math: out[b, c*FL + k, t] = x[b, c, t*HOP + k]  with FL=2048, HOP=512,
T = (S - FL)//HOP + 1 = 934.

Decomposition (k = 512*q + 128*i + p, q,i in [0,4), p in [0,128)):
    out[b, c*FL + 512q + 128i + p, t] = X[t+q, 128i+p]
where X[j, r] = x[b, c, j*512 + r] (j < 937). Per (b, c) this is one
937x512 -> 512x937 transpose; each of the 16 output row-blocks is a
contiguous column-slice XT[128i:128(i+1), q:q+934] written densely.

Schedule per core (one batch element per NeuronCore, 8-way data parallel):
  - HBM is the budget (~19.1 MB at ~390 GB/s aggregate over the three DMA
    dispatch rings). The kernel keeps every ring's FIFO non-empty from the
    first load to the last store so the HBM pipe never idles:
      * loads (2 KB descriptors) ride the gpsimd SWDGE ring, which streams
        them ~2.3x faster than a HWDGE ring (Q7 packs 64-desc packets);
        four small c0 pieces go to the idle HWDGE rings early, before the
        deep SWDGE queue can starve them of shared-engine time.
      * the 41-row remainder loads as a full 128-partition tail tile
        overlapping jc6 (a 41-descriptor DMA is chopped onto ~2 SDMA
        engines and its late semaphore would gate the whole first block).
      * transposes (TensorE via identity matmul, PSUM, DVE copies) chase
        the load stream in landing order; each block's 4 dense ~478 KB
        stores (3736 B/descriptor) alternate over the two HWDGE rings the
        moment the block is assembled, draining behind the c1 loads so
        HBM stays saturated from first load byte to final store.
  - The NEFF's fixed ~6.3 us full-semaphore-file clear epilogue + barriers
    and the ~2 us framework preamble are invariant; everything between is
    paced by HBM. Measured 63.1-63.9 us/core unthrottled; hardware
    activity-throttling adds 0-20 us on hot runs.
"""

import numpy as np

import concourse.mybir as mybir
import concourse.tile as tile
from concourse import bacc, bass_utils

B, C, S = 8, 2, 480000
FL, HOP = 2048, 512
T = (S - FL) // HOP + 1          # 934 frames
NQ = FL // HOP                   # 4 hop-shifts per frame length
NJ = T + NQ - 1                  # 937 hop-chunks of input actually used
P = 128
NI = HOP // P                    # 4 row-blocks of 128 within a hop
NJC_FULL = NJ // P               # 7 full 128-row chunks
NJ_REM = NJ - NJC_FULL * P       # 41 remainder rows
F32 = mybir.dt.float32

_NC_CACHE = None


def _emit(tc, nc, x, ident_in, out):
    # x: [C, S] f32 (this core's batch element), out: [C*FL, T] f32
    sy, sc, gp = nc.sync, nc.scalar, nc.gpsimd
    store_rr = [0]

    def store_dma(dst, src):
        eng = (sy, sc)[store_rr[0] & 1]
        store_rr[0] += 1
        eng.dma_start(dst, src)

    with tc.tile_pool(name="consts", bufs=1) as consts, \
         tc.tile_pool(name="loads", bufs=1) as loadp, \
         tc.tile_pool(name="xt", bufs=1) as xtp, \
         tc.tile_pool(name="ps", bufs=8, space="PSUM") as psp:
        ident = consts.tile([P, P], F32, name="ident")
        sy.dma_start(ident[:, :], ident_in[:, :])

        # Load layout: a_all[p, jc*HOP + r] = x[c, (jc*128 + p)*HOP + r]
        # (dense 2 KB rows per partition per jc chunk); a_tail covers the
        # last 128 hop-chunks j in [NJ-128, NJ), overlapping jc6.
        a_alls, a_tails = [], []
        for c in range(C):
            a_alls.append(
                loadp.tile([P, NJC_FULL * HOP], F32, name=f"a{c}", tag=f"a{c}")
            )
            a_tails.append(
                loadp.tile([P, HOP], F32, name=f"at{c}", tag=f"at{c}")
            )
        # The 41 remainder hop-chunks load as a full 128-partition tail
        # tile (overlapping jc6 — the duplicate read is 0.26 MB and keeps
        # every load a clean 128-descriptor DMA; a 41-descriptor DMA gets
        # chopped onto ~2 SDMA engines and its semaphore then gates the
        # whole first block). Most of c0 rides the SWDGE ring in jc order;
        # two small single-piece DMAs go to the HWDGE rings early enough
        # to finish before the deep SWDGE queue starves them (SWDGE's
        # 64-descriptor packets monopolize shared engines). c1 streams
        # behind c0 on the SWDGE ring only, keeping the HWDGE rings clear
        # for c0's stores.
        JT0 = NJ - P                           # tail covers j in [JT0, NJ)
        avs, xv_fulls = [], []
        for c in range(C):
            xv_fulls.append(
                x[c, 0:NJC_FULL * P * HOP].rearrange(
                    "(jc p r) -> p jc r", p=P, r=HOP
                )
            )
            avs.append(a_alls[c][:, :].rearrange("p (jc r) -> p jc r", r=HOP))

        def load(c, j0, j1, eng):
            eng.dma_start(avs[c][:, j0:j1], xv_fulls[c][:, j0:j1])

        def load_tail(c, eng):
            xv = x[c, 0:NJ * HOP].rearrange("(j r) -> j r", r=HOP)
            eng.dma_start(a_tails[c][:, :], xv[JT0:NJ])

        load_tail(0, gp)
        load(0, 0, 2, gp)
        load(0, 2, 3, gp)
        load(0, 3, 4, sy)
        load(0, 4, 5, sy)
        load(0, 5, 6, sc)
        load(0, 6, 7, sc)
        load_tail(1, gp)
        load(1, 0, 4, gp)
        load(1, 4, 7, gp)

        # Transpose + store. xt tiles are distinct per (c, i) so no reuse
        # dependencies gate the pipeline; each block's 4 dense ~478 KB
        # stores enqueue the moment its 8 PSUM->SBUF copies land.
        # Per-block transpose order chases load-landing order (for c0 the
        # sync/scalar pieces land first, then the tail, then the gp pieces;
        # for c1 the tail lands first, then jc0..6 stream in order).
        jc_order = {
            0: (3, 4, 5, 6, NJC_FULL, 0, 1, 2),
            1: (NJC_FULL, *range(NJC_FULL)),
        }
        for c in range(C):
            a_all, a_tail = a_alls[c], a_tails[c]
            for i in range(NI):
                xt = xtp.tile([P, NJ], F32, name=f"xt{c}{i}", tag=f"xt{c}{i}")
                for jc in jc_order[c]:
                    pt = psp.tile([P, P], F32, name="pt", tag="pt")
                    if jc < NJC_FULL:
                        j0, nj = jc * P, P
                        src = a_all[:, jc * HOP + i * P: jc * HOP + (i + 1) * P]
                        nc.tensor.transpose(pt[:, :nj], src, ident[:nj, :nj])
                        nc.vector.tensor_copy(xt[:, j0:j0 + nj], pt[:, :nj])
                    else:
                        # Remainder: transpose a 64-row slice (matmul base
                        # partition must be 0/32/64) and copy out the 41
                        # columns that land past jc6.
                        j0 = NJC_FULL * P
                        off = j0 - (JT0 + 64)
                        src = a_tail[64:P, i * P:(i + 1) * P]
                        nc.tensor.transpose(pt[:, :64], src, ident[64:P, 64:P])
                        nc.vector.tensor_copy(
                            xt[:, j0:NJ], pt[:, off:off + NJ_REM]
                        )
                for q in range(NQ):
                    base = c * FL + q * HOP + i * P
                    if c == C - 1 and i == NI - 1 and q == NQ - 1:
                        # Last store: split so the scalar ring (which
                        # otherwise trails by ~0.6 us) hands its tail to
                        # the sync ring and both drain out together.
                        ts = 760
                        store_dma(out[base:base + P, :ts], xt[:, q:q + ts])
                        store_dma(out[base:base + P, ts:], xt[:, q + ts:q + T])
                    else:
                        store_dma(out[base:base + P, :], xt[:, q:q + T])


def _build():
    nc = bacc.Bacc(
        "TRN2",
        target_bir_lowering=False,
        debug=False,
        enable_asserts=False,
        num_devices=B,
    )
    x = nc.dram_tensor("x", [C, S], F32, kind="ExternalInput").ap()
    ident_in = nc.dram_tensor("ident", [P, P], F32, kind="ExternalInput").ap()
    out = nc.dram_tensor("out", [C * FL, T], F32, kind="ExternalOutput").ap()
    with tile.TileContext(nc) as tc:
        _emit(tc, nc, x, ident_in, out)
    nc.compile()
    return nc


def _get_nc():
    global _NC_CACHE
    if _NC_CACHE is None:
        _NC_CACHE = _build()
    return _NC_CACHE


def make_in_maps(x):
    ident = np.eye(P, dtype=np.float32)
    return [
        {"x": np.ascontiguousarray(x[b]), "ident": ident} for b in range(B)
    ]


def kernel(**inputs):
    x = np.ascontiguousarray(np.asarray(inputs["x"]), dtype=np.float32)
    assert x.shape == (B, C, S), x.shape
    nc = _get_nc()
    res = bass_utils.run_bass_kernel_spmd(
        nc, make_in_maps(x), core_ids=list(range(B))
    )
    return np.stack([r["out"] for r in res.results], axis=0)
